# revision 15
# baseline (speedup 1.0000x reference)
"""BinConv (binarize-both-operands 3x3 conv, stride 1, pad 1) on 8 trn2 cores.

Strategy: data-parallel over batch (4 images per core), weights replicated.

Per-core device kernel:
  - x chunk DMA'd in as fp32, binarized with one exact DVE op
    (is_ge 0.0, subtract 0.5) -> {-0.5, +0.5} in fp8e4.
  - Weights arrive host-transposed as [c_in, tap, c_out] fp32, binarized on
    device to {-2, +2} fp8e4 (so x*w products are exactly +-1; PSUM fp32
    accumulation of <= 1152 such products is exact).
  - Conv = 9 shift-matmuls over a row-padded image buffer; the buffer has a
    zero row above/below and 1-element guards, and W-dim edge wrap errors are
    cancelled by 6 small correction matmuls per image + strided subtracts.
  - PSUM -> SBUF with bias via ACT activation(Copy, bias), then DMA out.
"""

import os
import sys

import numpy as np

for _p in ("/opt/trn_rl_repo", "/opt/pypackages"):
    if _p not in sys.path and os.path.isdir(_p):
        sys.path.append(_p)

from concourse import bacc, bass, mybir, tile  # noqa: E402
from concourse.ap import AP  # noqa: E402
from concourse.bass_utils import run_bass_kernel_spmd  # noqa: E402

F32 = mybir.dt.float32
F8 = mybir.dt.float8e4
ALU = mybir.AluOpType
ACTF = mybir.ActivationFunctionType

N_CORES = 8
P = 128  # C_in == C_out == partitions
H = W = 112
HWIMG = H * W  # 12544
IMGS = 4  # images per core
QROWS = 28  # rows per DMA chunk (quarter image)
CHUNK = QROWS * W  # 3136
NTILE = 448  # matmul free dim (4 output rows), one PSUM bank
TILES_PER_CHUNK = CHUNK // NTILE  # 7
# padded image buffer: [guard(1) | zero row (112) | 112 data rows | zero row | guard(1)]
TDATA = 113  # offset of data row 0
TSIZE = 1 + 114 * W + 1  # 12770
TZERO_TAIL = TDATA + HWIMG  # 12657

# tap t = (kh, kw); read offset of tap t for output col o is T[o + off(t)]
OFF = [(t // 3) * W + (t % 3) for t in range(9)]

# matmul variant: "A" = 9 single matmuls; "B" = 3 DoubleRow kh-pairs + 3
# singles (pair strides all %16==0); "C" = 4 DoubleRow lexicographic pairs +
# 1 single (pair strides 1/110/1/1).
VARIANT = os.environ.get("BINCONV_VARIANT", "C")


def _pair_ap(base: bass.AP, d: int, n: int) -> bass.AP:
    """[P, 2, n] view of a flat SBUF tile slice: pair elements d apart."""
    return AP(base.tensor, base.offset, [list(base.ap[0]), [d, 2], [1, n]])


def _weight_groups(variant):
    """Sequence of (taps, is_pair) weight sets covering all 9 taps."""
    if variant == "A":
        return [((t,), False) for t in range(9)]
    if variant == "B":
        return [((t, t + 3), True) for t in (0, 1, 2)] + [
            ((t,), False) for t in (6, 7, 8)
        ]
    if variant == "C":
        return [((2 * p, 2 * p + 1), True) for p in range(4)] + [((8,), False)]
    raise ValueError(variant)


def _emit_main_matmuls(nc, ps_list, wb2, T, o0_list, n, variant):
    """Accumulate all 9 taps into each PSUM tile in ps_list (one per o0).

    Loops weight-sets outermost so consecutive matmuls share the stationary
    operand (amortizes LDWEIGHTS across the tiles in the group).
    """
    dr = mybir.MatmulPerfMode.DoubleRow
    groups = _weight_groups(variant)
    for g, (taps, is_pair) in enumerate(groups):
        t = taps[0]
        if is_pair:
            step = taps[1] - taps[0]
            lhsT = wb2[:, t : t + step + 1 : step, :]
        else:
            lhsT = wb2[:, t, :]
        for ps, o0 in zip(ps_list, o0_list):
            base = T[:, o0 + OFF[t] : o0 + OFF[t] + n]
            if is_pair:
                rhs = _pair_ap(base, OFF[taps[1]] - OFF[t], n)
            else:
                rhs = base
            nc.tensor.matmul(
                ps[:, :n],
                lhsT,
                rhs,
                start=(g == 0),
                stop=(g == len(groups) - 1),
                perf_mode=dr if is_pair else None,
            )


def build(n_imgs=IMGS, variant=VARIANT, n_cores=N_CORES):
    nc = bacc.Bacc(
        "TRN2", target_bir_lowering=False, debug=False, num_devices=n_cores
    )
    x_ext = nc.declare_dram_parameter("x", [n_imgs, P, H, W], F32, isOutput=False)
    wt_ext = nc.declare_dram_parameter("wt", [P, 9, P], F32, isOutput=False)
    b_ext = nc.declare_dram_parameter("b", [P, 1], F32, isOutput=False)
    out_ext = nc.declare_dram_parameter("out", [n_imgs, P, H, W], F32, isOutput=True)

    with tile.TileContext(nc) as tc:
        with (
            tc.tile_pool(name="wpool", bufs=1) as wpool,
            tc.tile_pool(name="inpool", bufs=4) as inpool,
            tc.tile_pool(name="tpool", bufs=4) as tpool,
            tc.tile_pool(name="outpool", bufs=4) as outpool,
            tc.tile_pool(name="cpool", bufs=2) as cpool,
            tc.tile_pool(name="pspool", bufs=6, space="PSUM") as pspool,
            tc.tile_pool(name="cpsum", bufs=1, space="PSUM") as cpsum,
        ):
            # ---- weights / bias prep (one-time; DMA'd on the scalar ring so
            # the first x chunks head the sync ring) ----
            wt_stage = wpool.tile([P, 9 * P], F32)
            nc.scalar.dma_start(wt_stage[:], wt_ext[:])
            bias = wpool.tile([P, 1], F32)
            nc.scalar.dma_start(bias[:], b_ext[:])
            whalf = wpool.tile([P, 9 * P], F8)  # {-0.5, +0.5}
            nc.vector.tensor_scalar(
                whalf[:], wt_stage[:], 0.0, 0.5, ALU.is_ge, ALU.subtract
            )
            wb2 = wpool.tile([P, 9, P], F8)  # {-2, +2}
            nc.vector.tensor_scalar_mul(wb2[:], whalf[:].rearrange("p (t c) -> p t c", t=9), 4.0)

            for img in range(n_imgs):
                # ---- load + binarize into padded buffer ----
                T = tpool.tile([P, TSIZE], F8)
                nc.gpsimd.memset(T[:, 0:TDATA], 0.0)
                nc.gpsimd.memset(T[:, TZERO_TAIL:TSIZE], 0.0)
                for q in range(4):
                    xin = inpool.tile([P, CHUNK], F32)
                    # inputs ride two DMA queues (sync HWDGE + gpsimd SWDGE)
                    # for a 2:1 service edge over the output store queue; both
                    # issuing engines are free of compute-gated instructions
                    eng = nc.sync if (img * 4 + q) % 2 == 0 else nc.gpsimd
                    eng.dma_start(
                        xin[:], x_ext[img, :, q * QROWS : (q + 1) * QROWS, :]
                    )
                    nc.vector.tensor_scalar(
                        T[:, TDATA + q * CHUNK : TDATA + (q + 1) * CHUNK],
                        xin[:],
                        0.0,
                        0.5,
                        ALU.is_ge,
                        ALU.subtract,
                    )

                # ---- per-quarter: corrections + main conv tiles ----
                for q in range(4):
                    # W-edge wrap corrections for this quarter's 28 rows:
                    # main matmul at (r, w=0), tap (kh,0) wrongly reads
                    # T[(r+kh)*112]; at (r, w=111), tap (kh,2) wrongly reads
                    # T[(r+kh+1)*112 + 1]. Per-quarter so PE only ever
                    # depends on chunks q / q+1, never the whole image.
                    span = (QROWS - 1) * W + 1
                    cpsL = cpsum.tile([P, QROWS], F32)
                    for j, kh in enumerate(range(3)):
                        base = (q * QROWS + kh) * W
                        nc.tensor.matmul(
                            cpsL[:],
                            wb2[:, 3 * kh, :],
                            T[:, base : base + span : W],
                            start=(j == 0),
                            stop=(j == 2),
                        )
                    cpsR = cpsum.tile([P, QROWS], F32)
                    for j, kh in enumerate(range(3)):
                        base = (q * QROWS + kh + 1) * W + 1
                        nc.tensor.matmul(
                            cpsR[:],
                            wb2[:, 3 * kh + 2, :],
                            T[:, base : base + span : W],
                            start=(j == 0),
                            stop=(j == 2),
                        )
                    corrL = cpool.tile([P, QROWS], F32)
                    nc.vector.tensor_copy(corrL[:], cpsL[:])
                    corrR = cpool.tile([P, QROWS], F32)
                    nc.vector.tensor_copy(corrR[:], cpsR[:])

                    outsb = outpool.tile([P, CHUNK], F32)
                    for s0 in range(0, TILES_PER_CHUNK, 3):
                        snames = list(range(s0, min(s0 + 3, TILES_PER_CHUNK)))
                        ps_list = [
                            pspool.tile([P, NTILE], F32, name=f"ps{i}", tag="ps")
                            for i in range(len(snames))
                        ]
                        o0_list = [q * CHUNK + s * NTILE for s in snames]
                        _emit_main_matmuls(nc, ps_list, wb2, T, o0_list, NTILE, variant)
                        for ps, s in zip(ps_list, snames):
                            nc.scalar.activation(
                                outsb[:, s * NTILE : (s + 1) * NTILE],
                                ps[:],
                                ACTF.Identity,
                                bias=bias[:],
                            )
                    # subtract wrap corrections on edge columns
                    nc.vector.tensor_tensor(
                        outsb[:, 0:CHUNK:W],
                        outsb[:, 0:CHUNK:W],
                        corrL[:],
                        ALU.subtract,
                    )
                    nc.vector.tensor_tensor(
                        outsb[:, W - 1 : CHUNK : W],
                        outsb[:, W - 1 : CHUNK : W],
                        corrR[:],
                        ALU.subtract,
                    )
                    nc.scalar.dma_start(
                        out_ext[img, :, q * QROWS : (q + 1) * QROWS, :], outsb[:]
                    )

    nc.compile()
    return nc


def _host_prep(x, W_, b):
    x = np.ascontiguousarray(np.asarray(x, dtype=np.float32))
    W_ = np.asarray(W_, dtype=np.float32)
    b = np.asarray(b, dtype=np.float32)
    # [C_out, C_in, 3, 3] -> [C_in, tap, C_out] (pure layout change)
    wt = np.ascontiguousarray(np.transpose(W_, (1, 2, 3, 0)).reshape(P, 9, P))
    b2 = np.ascontiguousarray(b.reshape(P, 1))
    return x, wt, b2


def run(x, W, b, trace=False, variant=VARIANT):
    x, wt, b2 = _host_prep(x, W, b)
    n = x.shape[0]
    per = n // N_CORES
    nc = build(n_imgs=per, variant=variant)
    in_maps = [
        {"x": np.ascontiguousarray(x[k * per : (k + 1) * per]), "wt": wt, "b": b2}
        for k in range(N_CORES)
    ]
    res = run_bass_kernel_spmd(nc, in_maps, list(range(N_CORES)), trace=trace)
    out = np.concatenate([res.results[k]["out"] for k in range(N_CORES)], axis=0)
    return out, res


def kernel(x, W, b):
    out, _ = run(x, W, b, trace=False)
    return out


if __name__ == "__main__":
    xs = np.random.randn(32, P, H, W).astype(np.float32)
    Ws = np.random.randn(P, P, 3, 3).astype(np.float32) * 0.03
    bs = np.random.randn(P).astype(np.float32) * 0.01
    out = kernel(xs, Ws, bs)
    print(out.shape, out.dtype)


# revision 19
# speedup vs baseline: 1.1326x; 1.1326x over previous
"""BinConv (binarize-both-operands 3x3 conv, stride 1, pad 1) on 8 trn2 cores.

Strategy: data-parallel over batch (4 images per core), weights replicated.

Per-core device kernel:
  - x chunk DMA'd in as fp32, binarized with one exact DVE op
    (is_ge 0.0, subtract 0.5) -> {-0.5, +0.5} in fp8e4.
  - Weights arrive host-transposed as [c_in, tap, c_out] fp32, binarized on
    device to {-2, +2} fp8e4 (so x*w products are exactly +-1; PSUM fp32
    accumulation of <= 1152 such products is exact).
  - Conv = 9 shift-matmuls over a row-padded image buffer; the buffer has a
    zero row above/below and 1-element guards, and W-dim edge wrap errors are
    cancelled by 6 small correction matmuls per image + strided subtracts.
  - PSUM -> SBUF with bias via ACT activation(Copy, bias), then DMA out.
"""

import os
import sys

import numpy as np

for _p in ("/opt/trn_rl_repo", "/opt/pypackages"):
    if _p not in sys.path and os.path.isdir(_p):
        sys.path.append(_p)

from concourse import bacc, bass, mybir, tile  # noqa: E402
from concourse.ap import AP  # noqa: E402
from concourse.bass_utils import run_bass_kernel_spmd  # noqa: E402

F32 = mybir.dt.float32
F8 = mybir.dt.float8e4
ALU = mybir.AluOpType
ACTF = mybir.ActivationFunctionType

N_CORES = 8
P = 128  # C_in == C_out == partitions
H = W = 112
HWIMG = H * W  # 12544
IMGS = 4  # images per core
QROWS = 28  # rows per DMA chunk (quarter image)
CHUNK = QROWS * W  # 3136
NTILE = 448  # matmul free dim (4 output rows), one PSUM bank
TILES_PER_CHUNK = CHUNK // NTILE  # 7
# padded image buffer: [guard(1) | zero row (112) | 112 data rows | zero row | guard(1)]
TDATA = 113  # offset of data row 0
TSIZE = 1 + 114 * W + 1  # 12770
TZERO_TAIL = TDATA + HWIMG  # 12657

# tap t = (kh, kw); read offset of tap t for output col o is T[o + off(t)]
OFF = [(t // 3) * W + (t % 3) for t in range(9)]

# matmul variant: "A" = 9 single matmuls; "B" = 3 DoubleRow kh-pairs + 3
# singles (pair strides all %16==0); "C" = 4 DoubleRow lexicographic pairs +
# 1 single (pair strides 1/110/1/1).
VARIANT = os.environ.get("BINCONV_VARIANT", "C")


def _pair_ap(base: bass.AP, d: int, n: int) -> bass.AP:
    """[P, 2, n] view of a flat SBUF tile slice: pair elements d apart."""
    return AP(base.tensor, base.offset, [list(base.ap[0]), [d, 2], [1, n]])


def _weight_groups(variant):
    """Sequence of (taps, is_pair) weight sets covering all 9 taps."""
    if variant == "A":
        return [((t,), False) for t in range(9)]
    if variant == "B":
        return [((t, t + 3), True) for t in (0, 1, 2)] + [
            ((t,), False) for t in (6, 7, 8)
        ]
    if variant == "C":
        return [((2 * p, 2 * p + 1), True) for p in range(4)] + [((8,), False)]
    raise ValueError(variant)


def _emit_main_matmuls(nc, ps_list, wb2, T, o0_list, n, variant):
    """Accumulate all 9 taps into each PSUM tile in ps_list (one per o0).

    Loops weight-sets outermost so consecutive matmuls share the stationary
    operand (amortizes LDWEIGHTS across the tiles in the group).
    """
    dr = mybir.MatmulPerfMode.DoubleRow
    groups = _weight_groups(variant)
    for g, (taps, is_pair) in enumerate(groups):
        t = taps[0]
        if is_pair:
            step = taps[1] - taps[0]
            lhsT = wb2[:, t : t + step + 1 : step, :]
        else:
            lhsT = wb2[:, t, :]
        for ps, o0 in zip(ps_list, o0_list):
            base = T[:, o0 + OFF[t] : o0 + OFF[t] + n]
            if is_pair:
                rhs = _pair_ap(base, OFF[taps[1]] - OFF[t], n)
            else:
                rhs = base
            nc.tensor.matmul(
                ps[:, :n],
                lhsT,
                rhs,
                start=(g == 0),
                stop=(g == len(groups) - 1),
                perf_mode=dr if is_pair else None,
            )


def build(n_imgs=IMGS, variant=VARIANT, n_cores=N_CORES):
    nc = bacc.Bacc(
        "TRN2", target_bir_lowering=False, debug=False, num_devices=n_cores
    )
    x_ext = nc.declare_dram_parameter("x", [n_imgs, P, H, W], F32, isOutput=False)
    wt_ext = nc.declare_dram_parameter("wt", [P, 9, P], F32, isOutput=False)
    b_ext = nc.declare_dram_parameter("b", [P, 1], F32, isOutput=False)
    out_ext = nc.declare_dram_parameter("out", [n_imgs, P, H, W], F32, isOutput=True)

    with tile.TileContext(nc) as tc:
        with (
            tc.tile_pool(name="wpool", bufs=1) as wpool,
            tc.tile_pool(name="inpool", bufs=4) as inpool,
            tc.tile_pool(name="tpool", bufs=4) as tpool,
            tc.tile_pool(name="outpool", bufs=4) as outpool,
            tc.tile_pool(name="cpool", bufs=2) as cpool,
            tc.tile_pool(name="pspool", bufs=7, space="PSUM") as pspool,
            tc.tile_pool(name="cpsum", bufs=1, space="PSUM") as cpsum,
        ):
            # ---- weights / bias prep (one-time; DMA'd on the scalar ring so
            # the first x chunks head the sync ring) ----
            wt_stage = wpool.tile([P, 9 * P], F32)
            nc.scalar.dma_start(wt_stage[:], wt_ext[:])
            bias = wpool.tile([P, 1], F32)
            nc.scalar.dma_start(bias[:], b_ext[:])
            whalf = wpool.tile([P, 9 * P], F8)  # {-0.5, +0.5}
            nc.vector.tensor_scalar(
                whalf[:], wt_stage[:], 0.0, 0.5, ALU.is_ge, ALU.subtract
            )
            wb2 = wpool.tile([P, 9, P], F8)  # {-2, +2}
            nc.vector.tensor_scalar_mul(wb2[:], whalf[:].rearrange("p (t c) -> p t c", t=9), 4.0)

            for img in range(n_imgs):
                # ---- load + binarize into padded buffer ----
                T = tpool.tile([P, TSIZE], F8)
                nc.gpsimd.memset(T[:, 0:TDATA], 0.0)
                nc.gpsimd.memset(T[:, TZERO_TAIL:TSIZE], 0.0)
                for q in range(4):
                    xin = inpool.tile([P, CHUNK], F32)
                    # sync engine does nothing else -> input DMA dispatch is
                    # never gated behind compute in an engine FIFO
                    nc.sync.dma_start(
                        xin[:], x_ext[img, :, q * QROWS : (q + 1) * QROWS, :]
                    )
                    nc.vector.tensor_scalar(
                        T[:, TDATA + q * CHUNK : TDATA + (q + 1) * CHUNK],
                        xin[:],
                        0.0,
                        0.5,
                        ALU.is_ge,
                        ALU.subtract,
                    )

                # ---- per-quarter: corrections + main conv tiles ----
                for q in range(4):
                    # W-edge wrap corrections for this quarter's 28 rows:
                    # main matmul at (r, w=0), tap (kh,0) wrongly reads
                    # T[(r+kh)*112]; at (r, w=111), tap (kh,2) wrongly reads
                    # T[(r+kh+1)*112 + 1]. Per-quarter so PE only ever
                    # depends on chunks q / q+1, never the whole image.
                    span = (QROWS - 1) * W + 1
                    # both corrections share one PSUM bank (two accumulation
                    # groups at disjoint offsets) so mains get 7 banks
                    cps = cpsum.tile([P, 2 * QROWS], F32)
                    for j, kh in enumerate(range(3)):
                        base = (q * QROWS + kh) * W
                        nc.tensor.matmul(
                            cps[:, 0:QROWS],
                            wb2[:, 3 * kh, :],
                            T[:, base : base + span : W],
                            start=(j == 0),
                            stop=(j == 2),
                        )
                    for j, kh in enumerate(range(3)):
                        base = (q * QROWS + kh + 1) * W + 1
                        nc.tensor.matmul(
                            cps[:, QROWS : 2 * QROWS],
                            wb2[:, 3 * kh + 2, :],
                            T[:, base : base + span : W],
                            start=(j == 0),
                            stop=(j == 2),
                        )
                    corr = cpool.tile([P, 2 * QROWS], F32)
                    nc.vector.tensor_copy(corr[:], cps[:])

                    outsb = outpool.tile([P, CHUNK], F32)
                    for s0 in range(0, TILES_PER_CHUNK, TILES_PER_CHUNK):
                        snames = list(range(s0, TILES_PER_CHUNK))
                        ps_list = [
                            pspool.tile([P, NTILE], F32, name=f"ps{i}", tag="ps")
                            for i in range(len(snames))
                        ]
                        o0_list = [q * CHUNK + s * NTILE for s in snames]
                        _emit_main_matmuls(nc, ps_list, wb2, T, o0_list, NTILE, variant)
                        for ps, s in zip(ps_list, snames):
                            nc.scalar.activation(
                                outsb[:, s * NTILE : (s + 1) * NTILE],
                                ps[:],
                                ACTF.Identity,
                                bias=bias[:],
                            )
                    # subtract wrap corrections on edge columns
                    nc.vector.tensor_tensor(
                        outsb[:, 0:CHUNK:W],
                        outsb[:, 0:CHUNK:W],
                        corr[:, 0:QROWS],
                        ALU.subtract,
                    )
                    nc.vector.tensor_tensor(
                        outsb[:, W - 1 : CHUNK : W],
                        outsb[:, W - 1 : CHUNK : W],
                        corr[:, QROWS : 2 * QROWS],
                        ALU.subtract,
                    )
                    nc.scalar.dma_start(
                        out_ext[img, :, q * QROWS : (q + 1) * QROWS, :], outsb[:]
                    )

    nc.compile()
    return nc


def _host_prep(x, W_, b):
    x = np.ascontiguousarray(np.asarray(x, dtype=np.float32))
    W_ = np.asarray(W_, dtype=np.float32)
    b = np.asarray(b, dtype=np.float32)
    # [C_out, C_in, 3, 3] -> [C_in, tap, C_out] (pure layout change)
    wt = np.ascontiguousarray(np.transpose(W_, (1, 2, 3, 0)).reshape(P, 9, P))
    b2 = np.ascontiguousarray(b.reshape(P, 1))
    return x, wt, b2


def run(x, W, b, trace=False, variant=VARIANT):
    x, wt, b2 = _host_prep(x, W, b)
    n = x.shape[0]
    per = n // N_CORES
    nc = build(n_imgs=per, variant=variant)
    in_maps = [
        {"x": np.ascontiguousarray(x[k * per : (k + 1) * per]), "wt": wt, "b": b2}
        for k in range(N_CORES)
    ]
    res = run_bass_kernel_spmd(nc, in_maps, list(range(N_CORES)), trace=trace)
    out = np.concatenate([res.results[k]["out"] for k in range(N_CORES)], axis=0)
    return out, res


def kernel(x, W, b):
    out, _ = run(x, W, b, trace=False)
    return out


if __name__ == "__main__":
    xs = np.random.randn(32, P, H, W).astype(np.float32)
    Ws = np.random.randn(P, P, 3, 3).astype(np.float32) * 0.03
    bs = np.random.randn(P).astype(np.float32) * 0.01
    out = kernel(xs, Ws, bs)
    print(out.shape, out.dtype)
